# revision 9
# baseline (speedup 1.0000x reference)
"""Trainium2 Bass kernel for nn_BPANSequential (E/I balanced RNN scan).

Data-parallel over batch B=256 across 8 NeuronCores (32 batch rows per core).
All state is kept feature-on-partitions, batch-on-free ("transposed" layout):

  per step t, a combined [128, 64] block holds
    cols  0:32 : e-state / bal_e / r_e over the 32 batch rows (128 partitions)
    cols 32:64 : i-state / bal_i / r_i over the 32 batch rows (partitions 0:32)

Per time step the recurrent work is 4 small matmuls accumulating into a PSUM
block that was pre-filled with the input projection 0.2*(x_t @ W_x) by a
per-group "fill" matmul (start=True), so the DVE state update is a single
fused op  e_new = 0.8*e + psum  followed by one relu.  bal = 5*psum (weights
are pre-scaled by DT=0.2 on the host), evacuated per 8-step group on the
Scalar engine with the -b bias folded in.  The readout matmul streams the
relu chunk buffers through the PE once per 64-step chunk.
"""

import numpy as np

import concourse.bacc as bacc
import concourse.tile as tile
import concourse.mybir as mybir
from concourse.bass_utils import run_bass_kernel_spmd
from concourse.tile_rust import add_dep_helper

F32 = mybir.dt.float32
ALU = mybir.AluOpType

NCORE = 8
B_FULL, L_FULL, D = 256, 512, 128
NE, NI, C = 128, 32, 10
DT = 0.2
B = B_FULL // NCORE  # 32 batch rows per core

_BUILD_CACHE = {}


def _build(L, CH, GP, reps, with_bias, timing=False, mmdt=F32):
    """Build + compile the SPMD Bass module. Returns the compiled Bacc."""
    assert L % CH == 0 and CH % GP == 0
    ngroups = CH // GP
    nchunks = L // CH
    W2 = 2 * B  # 64: combined e|i block width

    nc = bacc.Bacc(
        "TRN2",
        target_bir_lowering=False,
        debug=False,
        enable_asserts=False,
        num_devices=NCORE,
    )

    if timing:
        xt = nc.dram_tensor("xt", [D, L * B], mmdt)
    else:
        xt = nc.dram_tensor("xt", [D, L * B], mmdt, kind="ExternalInput")
    w_ee = nc.dram_tensor("w_ee", [NE, NE], mmdt, kind="ExternalInput")
    w_ien = nc.dram_tensor("w_ien", [NI, NE], mmdt, kind="ExternalInput")
    w_ei = nc.dram_tensor("w_ei", [NE, NI], mmdt, kind="ExternalInput")
    w_iin = nc.dram_tensor("w_iin", [NI, NI], mmdt, kind="ExternalInput")
    w_xe = nc.dram_tensor("w_xe", [D, NE], mmdt, kind="ExternalInput")
    w_xi = nc.dram_tensor("w_xi", [D, NI], mmdt, kind="ExternalInput")
    w_ro = nc.dram_tensor("w_ro", [NE, C], mmdt, kind="ExternalInput")
    nbe = nc.dram_tensor("nbe", [NE, 1], F32, kind="ExternalInput")  # -b_e
    nbi = nc.dram_tensor("nbi", [NI, 1], F32, kind="ExternalInput")  # -b_i
    bro = nc.dram_tensor("bro", [C, 1], F32, kind="ExternalInput")  # b_ro
    if with_bias:
        be2 = nc.dram_tensor("be2", [1, NE], F32, kind="ExternalInput")  # 0.2*b_e
        bi2 = nc.dram_tensor("bi2", [1, NI], F32, kind="ExternalInput")  # 0.2*b_i

    if timing:
        o_r = nc.dram_tensor("r", [128, L * W2], mmdt)
        o_be = nc.dram_tensor("bale", [NE, L * B], F32)
        o_bi = nc.dram_tensor("bali", [NI, L * B], F32)
        o_lg = nc.dram_tensor("lg", [C, L * B], F32)
        o_dummy = nc.dram_tensor("tmark", [1, 4], F32, kind="ExternalOutput")
    else:
        o_r = nc.dram_tensor("r", [128, L * W2], mmdt, kind="ExternalOutput")
        o_be = nc.dram_tensor("bale", [NE, L * B], F32, kind="ExternalOutput")
        o_bi = nc.dram_tensor("bali", [NI, L * B], F32, kind="ExternalOutput")
        o_lg = nc.dram_tensor("lg", [C, L * B], F32, kind="ExternalOutput")

    with tile.TileContext(nc) as tc:
        with (
            tc.tile_pool(name="wpool", bufs=1) as wpool,
            tc.tile_pool(name="xpool", bufs=2) as xpool,
            tc.tile_pool(name="epool", bufs=2) as epool,
            tc.tile_pool(name="rpool", bufs=2) as rpool,
            tc.tile_pool(name="bepool", bufs=2) as bepool,
            tc.tile_pool(name="bipool", bufs=2) as bipool,
            tc.tile_pool(name="lgpool", bufs=2) as lgpool,
            tc.tile_pool(name="pspool", bufs=4, space="PSUM") as pspool,
            tc.tile_pool(name="lgps", bufs=2, space="PSUM") as lgps,
        ):
            # --- weights into SBUF once ---
            sb_ee = wpool.tile([NE, NE], mmdt)
            sb_ien = wpool.tile([NI, NE], mmdt)
            sb_ei = wpool.tile([NE, NI], mmdt)
            sb_iin = wpool.tile([NI, NI], mmdt)
            sb_xe = wpool.tile([D, NE], mmdt)
            sb_xi = wpool.tile([D, NI], mmdt)
            sb_ro = wpool.tile([NE, C], mmdt)
            sb_nbe = wpool.tile([NE, 1], F32)
            sb_nbi = wpool.tile([NI, 1], F32)
            sb_bro = wpool.tile([C, 1], F32)
            for dst, src in (
                (sb_ee, w_ee), (sb_ien, w_ien), (sb_ei, w_ei), (sb_iin, w_iin),
                (sb_xe, w_xe), (sb_xi, w_xi), (sb_ro, w_ro),
                (sb_nbe, nbe), (sb_nbi, nbi), (sb_bro, bro),
            ):
                nc.sync.dma_start(dst[:], src.ap())
            if with_bias:
                sb_be2 = wpool.tile([1, NE], F32)
                sb_bi2 = wpool.tile([1, NI], F32)
                sb_ones = wpool.tile([1, GP * B], F32)
                nc.sync.dma_start(sb_be2[:], be2.ap())
                nc.sync.dma_start(sb_bi2[:], bi2.ap())
                nc.vector.memset(sb_ones[:], 1.0)

            def body():
                prev_e = None
                prev_r = None
                for c in range(nchunks):
                    ech = epool.tile([128, (CH + 1) * W2], F32, tag="ech")
                    rch = rpool.tile([128, (CH + 1) * W2], mmdt, tag="rch")
                    if c == 0:
                        nc.vector.memset(ech[:, 0:W2], 0.0)
                        nc.vector.tensor_copy(rch[:, 0:W2], ech[:, 0:W2])
                    else:
                        nc.vector.tensor_copy(
                            ech[:, 0:W2], prev_e[:, CH * W2 : (CH + 1) * W2]
                        )
                        nc.vector.tensor_copy(
                            rch[:, 0:W2], prev_r[:, CH * W2 : (CH + 1) * W2]
                        )
                    xtc = xpool.tile([D, CH * B], mmdt, tag="xtc")
                    nc.sync.dma_start(xtc[:], xt.ap()[:, c * CH * B : (c + 1) * CH * B])
                    be_ch = bepool.tile([NE, CH * B], F32, tag="bech")
                    bi_ch = bipool.tile([NI, CH * B], F32, tag="bich")
                    lg_ch = lgpool.tile([C, CH * B], F32, tag="lgch")

                    for g in range(ngroups):
                        pt = pspool.tile([128, GP * W2], F32, tag="pt")
                        ptv = pt[:].rearrange("p (t w) -> p t w", w=W2)
                        xg = xtc[:, g * GP * B : (g + 1) * GP * B]
                        # fill: psum(t-block) = 0.2 * x_t @ W_x (+ 0.2*b)
                        m_e = nc.tensor.matmul(
                            ptv[:, :, 0:B], sb_xe[:], xg,
                            start=True, stop=False, skip_group_check=True,
                        )
                        m_i = nc.tensor.matmul(
                            ptv[0:NI, :, B:W2], sb_xi[:], xg,
                            start=False, stop=False, skip_group_check=True,
                        )
                        # the start=True bank clear must precede the i fill
                        add_dep_helper(m_i.ins, m_e.ins, sync=False,
                                       reason="psum fill order")
                        if with_bias:
                            nc.tensor.matmul(
                                ptv[:, :, 0:B], sb_be2[:], sb_ones[:],
                                start=False, stop=False, skip_group_check=True,
                            )
                            nc.tensor.matmul(
                                ptv[0:NI, :, B:W2], sb_bi2[:], sb_ones[:],
                                start=False, stop=False, skip_group_check=True,
                            )
                        for tl in range(GP):
                            tb = g * GP + tl  # block index within chunk
                            rp = rch[:, tb * W2 : (tb + 1) * W2]
                            ep = ech[:, tb * W2 : (tb + 1) * W2]
                            ps = pt[:, tl * W2 : (tl + 1) * W2]
                            nc.tensor.matmul(
                                ps[:, 0:B], sb_ee[:], rp[:, 0:B],
                                start=False, stop=False, skip_group_check=True,
                            )
                            nc.tensor.matmul(
                                ps[:, 0:B], sb_ien[:], rp[0:NI, B:W2],
                                start=False, stop=False, skip_group_check=True,
                            )
                            nc.tensor.matmul(
                                ps[0:NI, B:W2], sb_ei[:], rp[:, 0:B],
                                start=False, stop=False, skip_group_check=True,
                            )
                            nc.tensor.matmul(
                                ps[0:NI, B:W2], sb_iin[:], rp[0:NI, B:W2],
                                start=False, stop=False, skip_group_check=True,
                            )
                            en = ech[:, (tb + 1) * W2 : (tb + 2) * W2]
                            nc.vector.scalar_tensor_tensor(
                                en, ep, 0.8, ps, op0=ALU.mult, op1=ALU.add
                            )
                            nc.vector.tensor_scalar_max(
                                rch[:, (tb + 1) * W2 : (tb + 2) * W2], en, 0.0
                            )
                        # bal = 5*psum - b, evacuated per group on ScalarE
                        nc.scalar.activation(
                            out=be_ch[:, g * GP * B : (g + 1) * GP * B]
                            .rearrange("p (t b) -> p t b", b=B),
                            in_=ptv[:, :, 0:B],
                            func=mybir.ActivationFunctionType.Identity,
                            bias=sb_nbe[:], scale=5.0,
                        )
                        nc.scalar.activation(
                            out=bi_ch[:, g * GP * B : (g + 1) * GP * B]
                            .rearrange("p (t b) -> p t b", b=B),
                            in_=ptv[0:NI, :, B:W2],
                            func=mybir.ActivationFunctionType.Identity,
                            bias=sb_nbi[:], scale=5.0,
                        )

                    # readout: logits = W_ro.T @ r_e (+ b_ro), chunk at a time
                    rv = rch[:].rearrange("p (t w) -> p t w", w=W2)
                    TPB = 512 // B  # timesteps per logits matmul (N=512)
                    for s in range(CH // TPB):
                        lp = lgps.tile([C, 512], F32, tag="lp")
                        nc.tensor.matmul(
                            lp[:],
                            sb_ro[:],
                            rv[:, 1 + s * TPB : 1 + (s + 1) * TPB, 0:B],
                            start=True, stop=True, skip_group_check=True,
                        )
                        nc.scalar.activation(
                            out=lg_ch[:, s * 512 : (s + 1) * 512],
                            in_=lp[:],
                            func=mybir.ActivationFunctionType.Identity,
                            bias=sb_bro[:], scale=1.0,
                        )

                    nc.sync.dma_start(
                        o_r.ap()[:, c * CH * W2 : (c + 1) * CH * W2],
                        rch[:, W2 : (CH + 1) * W2],
                    )
                    nc.sync.dma_start(
                        o_be.ap()[:, c * CH * B : (c + 1) * CH * B], be_ch[:]
                    )
                    nc.sync.dma_start(
                        o_bi.ap()[:, c * CH * B : (c + 1) * CH * B], bi_ch[:]
                    )
                    nc.sync.dma_start(
                        o_lg.ap()[:, c * CH * B : (c + 1) * CH * B], lg_ch[:]
                    )
                    prev_e, prev_r = ech, rch

            if reps > 1:
                with tc.For_i(0, reps, 1):
                    body()
            else:
                body()
            if timing:
                dtile = wpool.tile([1, 4], F32)
                nc.vector.memset(dtile[:], 1.0)
                nc.sync.dma_start(o_dummy.ap(), dtile[:])

    nc.compile()
    return nc


def _prep_core_inputs(x_core, W, with_bias, np_mm):
    """x_core: [B, L, D] for this core -> in_map dict."""
    L = x_core.shape[1]
    xt = np.ascontiguousarray(x_core.transpose(2, 1, 0)).reshape(D, L * B).astype(np_mm)
    m = {"xt": xt}
    m.update(W)
    return m


def kernel(x_seq, W_xe, W_xi, W_ee, W_ei, W_ie, W_ii, b_e, b_i, W_ro, b_ro,
           L=None, reps=1, _time_only=False, mmdt_name="r32"):
    mmdt = {"f32": F32, "r32": mybir.dt.float32r, "bf16": mybir.dt.bfloat16}[mmdt_name]
    import ml_dtypes
    np_mm = np.float32 if mmdt_name in ("f32", "r32") else ml_dtypes.bfloat16
    x_seq = np.asarray(x_seq, np.float32)
    if L is None:
        L = x_seq.shape[1]
    CH = 64 if L % 64 == 0 else L
    GP = 8
    with_bias = bool(np.any(np.asarray(b_e)) or np.any(np.asarray(b_i)))

    timing = bool(_time_only)
    key = (L, CH, GP, reps, with_bias, timing, mmdt_name)
    if key not in _BUILD_CACHE:
        _BUILD_CACHE[key] = _build(L, CH, GP, reps, with_bias, timing, mmdt)
    nc = _BUILD_CACHE[key]

    f = np.float32
    W = {
        "w_ee": np.ascontiguousarray(DT * np.abs(W_ee), f).astype(np_mm),
        "w_ien": np.ascontiguousarray(-DT * np.abs(W_ie), f).astype(np_mm),
        "w_ei": np.ascontiguousarray(DT * np.abs(W_ei), f).astype(np_mm),
        "w_iin": np.ascontiguousarray(-DT * np.abs(W_ii), f).astype(np_mm),
        "w_xe": np.ascontiguousarray(DT * np.asarray(W_xe), f).astype(np_mm),
        "w_xi": np.ascontiguousarray(DT * np.asarray(W_xi), f).astype(np_mm),
        "w_ro": np.ascontiguousarray(W_ro, f).astype(np_mm),
        "nbe": np.ascontiguousarray(-np.asarray(b_e, f).reshape(NE, 1)),
        "nbi": np.ascontiguousarray(-np.asarray(b_i, f).reshape(NI, 1)),
        "bro": np.ascontiguousarray(np.asarray(b_ro, f).reshape(C, 1)),
    }
    if with_bias:
        W["be2"] = np.ascontiguousarray(DT * np.asarray(b_e, f).reshape(1, NE))
        W["bi2"] = np.ascontiguousarray(DT * np.asarray(b_i, f).reshape(1, NI))

    if timing:
        in_maps = [dict(W) for _ in range(NCORE)]
        res = run_bass_kernel_spmd(nc, in_maps, core_ids=list(range(NCORE)))
        return None
    in_maps = [
        _prep_core_inputs(x_seq[c * B : (c + 1) * B, :L], W, with_bias, np_mm)
        for c in range(NCORE)
    ]
    res = run_bass_kernel_spmd(nc, in_maps, core_ids=list(range(NCORE)))

    W2 = 2 * B
    logits_seq = np.empty((B_FULL, L, C), np.float32)
    r_e_seq = np.empty((B_FULL, L, NE), np.float32)
    r_i_seq = np.empty((B_FULL, L, NI), np.float32)
    bal_e_seq = np.empty((B_FULL, L, NE), np.float32)
    bal_i_seq = np.empty((B_FULL, L, NI), np.float32)
    for c in range(NCORE):
        out = res.results[c]
        sl = slice(c * B, (c + 1) * B)
        r = np.asarray(out["r"], np.float32).reshape(128, L, W2)
        r_e_seq[sl] = r[:, :, 0:B].transpose(2, 1, 0)
        r_i_seq[sl] = r[0:NI, :, B:W2].transpose(2, 1, 0)
        bal_e_seq[sl] = out["bale"].reshape(NE, L, B).transpose(2, 1, 0)
        bal_i_seq[sl] = out["bali"].reshape(NI, L, B).transpose(2, 1, 0)
        logits_seq[sl] = out["lg"].reshape(C, L, B).transpose(2, 1, 0)
    return (
        logits_seq[:, -1, :].copy(),
        logits_seq,
        r_e_seq,
        r_i_seq,
        bal_e_seq,
        bal_i_seq,
    )
